# revision 51
# baseline (speedup 1.0000x reference)
"""2-layer GAT on 8 Trainium2 NeuronCores (Bass/Tile).

Sharding: the 391 dst 128-node blocks are sorted by half-A edge count and
dealt in groups of 8 to the cores (one block per core per iteration), so the
per-iteration cross-core tile maximum stays near the mean.  Edges are routed
to the core owning their dst block; each block's edges occupy disjoint
half-A / half-B tile ranges (table halves keep dma_gather indices within
int16), so the two gathers per block never overlap and pipeline freely.

Per-layer device program (phase B only; projections are fused elsewhere):
  per 128-dst block: dma_gather A + B from the HBM row table
  [h | asrc f32-bits], one-hot dst masks on DVE (pair-packed 2x),
  w = exp(prelu(asrc+adst)) on ACT, weighted rows on DVE, aggregation +
  softmax denominator via PSUM-accumulated matmuls, epilogue scaling on ACT.

Per-edge adst: layer 1 expands its own via maskT matmuls (PSUM-resident);
the transposed masks come from PE-transposing the mk masks (no dst-local
replication broadcast), and the same masks also expand layer 2's adst (from
the fused [h2|asrc2|adst2] projection), written to HBM so layer 2 runs with
no pre-pass at all.

Launch 0 projects [h | asrc | adst] per node; layer 1's epilogue fuses the
layer-2 projection, so neither layer loads x.  Shards are exchanged through
the host between launches.
"""

import os
import numpy as np
import ml_dtypes

import concourse.bass as bass
import concourse.bacc as bacc
import concourse.tile as tile
from concourse import mybir
from concourse.bass_utils import run_bass_kernel_spmd

BF16 = ml_dtypes.bfloat16

N = 50000
E = 800000
IN = 128
H1 = 4
F1 = 64
NEG = 0.2
P = 128
NCORES = 8
NB = 49                 # block iterations per core
SHARD = NB * P          # 6272 rows per core in the table
NPAD = 391 * P          # 50048 padded node count
NGB = NCORES * NB       # 392 block slots (391 real + 1 dummy)
SPLIT = 196 * P         # 25088: gather-table half boundary (int16 idx limit)
GRP = 16                # proj-launch load group

_prog_cache = {}


# ----------------------------------------------------------------------------
# host-side edge preprocessing (shared by both layers)
# ----------------------------------------------------------------------------

def _prep_edges(edge_index):
    src = np.concatenate([edge_index[0].astype(np.int64), np.arange(N, dtype=np.int64)])
    dst = np.concatenate([edge_index[1].astype(np.int64), np.arange(N, dtype=np.int64)])
    order = np.argsort(dst, kind="stable")
    s = src[order]
    d = dst[order]

    gb = d >> 7                                   # global 128-block of dst
    cnt = np.bincount(gb, minlength=NGB)
    starts = np.concatenate([[0], np.cumsum(cnt)])
    isB = s >= SPLIT
    cntA = np.zeros(NGB, np.int64)
    for g in range(NGB):
        cntA[g] = np.count_nonzero(~isB[starts[g]:starts[g + 1]])

    # deal blocks sorted by half-A count: iteration i gets ranks [8i, 8i+8),
    # then anneal block swaps between iterations to shrink the per-iteration
    # cross-core tile maxima (the SPMD padding)
    blk_order = np.argsort(-cntA, kind="stable")
    asg = blk_order.reshape(NB, NCORES).copy()    # [iter, core] -> global block
    cntBg = cnt - cntA

    def _iter_cost(rows):
        a = cntA[rows].max()
        b = cntBg[rows].max()
        ta = -(-a // P)
        return ta + max(-(-b // P), 1 - ta)

    icost = np.array([_iter_cost(asg[i]) for i in range(NB)], np.int64)
    rng = np.random.default_rng(7)
    ii = rng.integers(0, NB, 1200000)
    jj = rng.integers(0, NB, 1200000)
    mm = rng.integers(0, NCORES, 1200000)
    nn = rng.integers(0, NCORES, 1200000)
    for k in range(1200000):
        i, j, m, n = ii[k], jj[k], mm[k], nn[k]
        if i == j:
            continue
        asg[i, m], asg[j, n] = asg[j, n], asg[i, m]
        ci, cj = _iter_cost(asg[i]), _iter_cost(asg[j])
        if ci + cj <= icost[i] + icost[j]:
            icost[i], icost[j] = ci, cj
        else:
            asg[i, m], asg[j, n] = asg[j, n], asg[i, m]

    nA = cntA[asg]                                # [NB, NCORES]
    nBc = cntBg[asg]
    nAmax = nA.max(1)                             # [NB]
    nBmax = nBc.max(1)
    # disjoint A/B tile ranges: no write overlap between the two gathers,
    # so they pipeline freely (a shared boundary tile would serialize them)
    TA = -(-nAmax // P)
    TB = -(-nBmax // P)
    TB = np.maximum(TB, 1 - TA)                   # dummy block: >= 1 tile
    Tm = (TA + TB).astype(np.int64)
    niA16 = (TA * P).astype(np.int64)             # static gather-A num_idxs
    fA = TA                                       # A tile count
    rA = np.zeros(NB, np.int64)
    niB = (TB * P).astype(np.int64)               # static gather-B num_idxs

    toff = np.zeros(NB + 1, np.int64)
    np.cumsum(Tm, out=toff[1:])
    Ttot = int(toff[NB])
    TMX = int(Tm.max())

    scol = np.zeros(NB + 1, np.int64)             # idx column offsets (per 16)
    np.cumsum(niA16 // 16 + niB // 16, out=scol[1:])
    Stot = int(scol[NB])

    idx_all = np.zeros((NCORES, P, Stot), np.int16)
    dstl = np.full((NCORES, Ttot, P), -1.0, np.float32)   # [t, p] layout

    for i in range(NB):
        sA = int(scol[i])
        sB = sA + int(niA16[i] // 16)
        for m in range(NCORES):
            g = asg[i, m]
            e0, e1 = starts[g], starts[g + 1]
            sb = s[e0:e1]
            mB = isB[e0:e1]
            shA = sb[~mB]
            shB = sb[mB] - SPLIT
            dlA = (d[e0:e1][~mB] - (g << 7)).astype(np.float32)
            dlB = (d[e0:e1][mB] - (g << 7)).astype(np.float32)
            na, nb_ = len(shA), len(shB)
            # gather-A idx: real | dummy-0 pad to its tile range
            ia = np.zeros(int(niA16[i]), np.int16)
            ia[:na] = shA
            # gather-B idx: real | dummy-0 pad to its tile range
            ib = np.zeros(int(niB[i]), np.int16)
            ib[:nb_] = shB
            for seg, off in ((ia, sA), (ib, sB)):
                if len(seg) == 0:
                    continue
                w = seg.reshape(-1, 16).T          # [16, S]
                idx_all[m][:, off:off + w.shape[1]] = np.tile(w, (8, 1))
            dl = np.full(int(Tm[i]) * P, -1.0, np.float32)
            dl[:na] = dlA
            boff = int(niA16[i])
            dl[boff:boff + nb_] = dlB
            dstl[m][toff[i]:toff[i] + Tm[i], :] = dl.reshape(int(Tm[i]), P)

    dstl_pt = np.ascontiguousarray(dstl.transpose(0, 2, 1))   # [m, P, Ttot]
    dstl_row = np.full((NCORES, NB, TMX * P), -1.0, np.float32)
    for m in range(NCORES):
        for i in range(NB):
            T = int(Tm[i])
            dstl_row[m, i, :T * P] = dstl[m, toff[i]:toff[i] + T].reshape(-1)
    dstl_row = dstl_row.astype(BF16)

    meta = dict(Tm=Tm.tolist(), toff=toff.tolist(), fA=fA.tolist(),
                rA=rA.tolist(), niA16=niA16.tolist(), niB=niB.tolist(),
                scol=scol.tolist(), Ttot=Ttot, Stot=Stot, Tmax=TMX,
                asg=asg.tolist())
    return meta, idx_all, dstl_pt, dstl_row


# ----------------------------------------------------------------------------
# launch 0: project own shard -> [h | asrc | adst] table slice
# ----------------------------------------------------------------------------

def _build_proj():
    dt = mybir.dt
    GRPP = int(os.environ.get("GAT_GRP0", "8"))
    KCH, H = 1, H1
    COUT = H1 * F1
    RC = COUT + 2 * H                   # [h | asrc | adst]
    OCOL = COUT + 4 * H                 # bf16 slots: h | asrc bits | adst bits
    nc = bacc.Bacc("TRN2", target_bir_lowering=False, debug=False,
                   num_devices=NCORES)
    xs = nc.dram_tensor("xs", [KCH, P, NB, P], dt.bfloat16,
                        kind="ExternalInput")
    wr = nc.dram_tensor("wr", [KCH, P, RC], dt.bfloat16,
                        kind="ExternalInput")
    hts = nc.dram_tensor("hts", [SHARD, OCOL], dt.bfloat16,
                         kind="ExternalOutput")
    with tile.TileContext(nc) as tc:
        with (
            tc.tile_pool(name="const", bufs=1) as cp,
            tc.tile_pool(name="pa", bufs=3) as pa,
            tc.tile_pool(name="psA", bufs=3, space="PSUM") as psA,
        ):
            wr_sb = cp.tile([P, KCH, RC], dt.bfloat16)
            nc.sync.dma_start(wr_sb[:], wr[:].rearrange("k p c -> p k c"))
            for g0 in range(0, NB, GRPP):
                gn = min(GRPP, NB - g0)
                xa = pa.tile([P, KCH, gn, P], dt.bfloat16, tag="xa")
                nc.sync.dma_start(
                    xa[:], xs[:, :, g0:g0 + gn, :].rearrange(
                        "k f t n -> f k t n"))
                hst = pa.tile([P, gn, OCOL], dt.bfloat16, tag="hst")
                for t0 in range(0, gn, 2):
                    pn = min(2, gn - t0)
                    ps = psA.tile([P, 2, 512], dt.float32, tag="psa")
                    for t2 in range(pn):
                        for k in range(KCH):
                            nc.tensor.matmul(ps[:, t2, 0:RC],
                                             lhsT=xa[:, k, t0 + t2, :],
                                             rhs=wr_sb[:, k, 0:RC],
                                             start=(k == 0),
                                             stop=(k == KCH - 1))
                    nc.scalar.activation(
                        hst[:, t0:t0 + pn, 0:COUT], ps[:, 0:pn, 0:COUT],
                        mybir.ActivationFunctionType.Copy)
                    nc.vector.tensor_copy(
                        hst[:, t0:t0 + pn, COUT:OCOL].bitcast(dt.float32),
                        ps[:, 0:pn, COUT:COUT + 2 * H])
                nc.sync.dma_start(
                    hts[g0 * P:(g0 + gn) * P, :].rearrange(
                        "(t n) c -> n t c", t=gn),
                    hst[:])
    nc.compile()
    return nc


# ----------------------------------------------------------------------------
# per-layer message-passing program (phase B)
# ----------------------------------------------------------------------------

def _build_layer(meta, layer, zero_bias):
    """layer 1: heads 4, F 64, fused layer-2 row production, no dense out.
    layer 2: heads 1, F 64, out f32 [SHARD, 64]."""
    dt = mybir.dt
    Tm, toff, fAm = meta["Tm"], meta["toff"], meta["fA"]
    niA16, niB, scol = meta["niA16"], meta["niB"], meta["scol"]
    Ttot, Stot, Tmax = meta["Ttot"], meta["Stot"], meta["Tmax"]

    if layer == 1:
        H, F = H1, F1
    else:
        H, F = 1, F1
    COUT = H * F
    AGC = COUT + H                    # aggregation psum cols: [num | den]
    TABC = 384 if layer == 1 else 128  # table row slots (256B granules)

    nc = bacc.Bacc("TRN2", target_bir_lowering=False, debug=False,
                   num_devices=NCORES)

    if layer == 1:
        w2r = nc.dram_tensor("w2r", [2, P, 66], dt.bfloat16,
                             kind="ExternalInput")
        identT = nc.dram_tensor("identT", [P, P], dt.bfloat16,
                                kind="ExternalInput")
        outT2 = nc.dram_tensor("outT2", [SHARD, 68], dt.bfloat16,
                               kind="ExternalOutput")
        # per-edge layer-2 adst, computed here by reusing the mT masks
        outA2 = nc.dram_tensor("outA2", [P, Ttot], dt.float32,
                               kind="ExternalOutput")
        dstlR = nc.dram_tensor("dstlR", [NB, Tmax * P], dt.bfloat16,
                               kind="ExternalInput")
        adstT = nc.dram_tensor("adstT", [P, NB * H], dt.bfloat16,
                               kind="ExternalInput")
    else:
        outT = nc.dram_tensor("outT", [SHARD, COUT], dt.float32,
                              kind="ExternalOutput")
        adsteT = nc.dram_tensor("adsteT", [P, Ttot], dt.float32,
                                kind="ExternalInput")
    idxT = nc.dram_tensor("idxT", [P, Stot], dt.int16, kind="ExternalInput")
    dstlT = nc.dram_tensor("dstlT", [P, Ttot], dt.bfloat16, kind="ExternalInput")
    brow = nc.dram_tensor("brow", [1, COUT], dt.float32, kind="ExternalInput")
    iot_r = nc.dram_tensor("iot_r", [1, P], dt.bfloat16, kind="ExternalInput")
    iot_c = nc.dram_tensor("iot_c", [P, 1], dt.float32, kind="ExternalInput")
    htab = nc.dram_tensor("htab", [NPAD, TABC], dt.bfloat16,
                          kind="ExternalInput")

    SP = bool(int(os.environ.get("GAT_SP", "0")))
    PBB = int(os.environ.get("GAT_PBB", "5"))
    PPK = int(os.environ.get("GAT_PPK", "3"))
    PBM = int(os.environ.get("GAT_PB_MOD%d" % layer,
                             "0" if layer == 1 else "2"))
    PBMI = int(os.environ.get("GAT_PBI%d" % layer, "0"))  # 1/K blocks on Pool
    GLB = int(os.environ.get("GAT_GLB", "3" if layer == 1 else "2"))
    GLA = int(os.environ.get("GAT_GLA", "2" if layer == 1 else "1"))
    SMB = int(os.environ.get("GAT_SMB", "4"))
    ALATE = bool(int(os.environ.get("GAT_ALATE", "1")))
    HPS = int(os.environ.get("GAT_HPS", "2"))      # hp split count
    EPL = bool(int(os.environ.get("GAT_EPL%d" % layer, "0")))  # epilogue after hp

    with tile.TileContext(nc) as tc:
        with (
            tc.tile_pool(name="const", bufs=1) as cp,
            tc.tile_pool(name="keep", bufs=1) as kp,
            tc.tile_pool(name="pp", bufs=3) as ppool,
            tc.tile_pool(name="pb", bufs=PBB) as pb,
            tc.tile_pool(name="sm", bufs=SMB) as sm,
            tc.tile_pool(name="psA", bufs=1, space="PSUM") as psA,
            tc.tile_pool(name="psB", bufs=2, space="PSUM") as psB,
            tc.tile_pool(name="psD", bufs=min(PPK - 1, GLA) + 1,
                         space="PSUM") as psD,
        ):
            # ---- resident constants ----
            b_sb = cp.tile([P, COUT], dt.float32)
            nc.sync.dma_start(b_sb[:], brow[:].broadcast_to([P, COUT]))
            ior_sb = cp.tile([P, P], dt.bfloat16)
            nc.sync.dma_start(ior_sb[:], iot_r[:].broadcast_to([P, P]))
            ioc_sb = cp.tile([P, 1], dt.float32)
            nc.sync.dma_start(ioc_sb[:], iot_c[:])
            if layer == 1:
                w2_sb = cp.tile([P, 2, 66], dt.bfloat16)
                nc.sync.dma_start(w2_sb[:], w2r[:].rearrange("k p c -> p k c"))
                id_sb = cp.tile([P, P], dt.bfloat16)
                nc.sync.dma_start(id_sb[:], identT[:])
            idx_sb = kp.tile([P, Stot], dt.int16)
            nc.sync.dma_start(idx_sb[:], idxT[:])
            dstl_sb = kp.tile([P, Ttot], dt.bfloat16)
            nc.sync.dma_start(dstl_sb[:], dstlT[:])
            if layer == 1:
                adst_sh = kp.tile([P, NB * H], dt.bfloat16)
                nc.sync.dma_start(adst_sh[:], adstT[:])
                a2all = kp.tile([P, Ttot], dt.float32)
            else:
                adste_sb = kp.tile([P, Ttot], dt.float32)
                nc.sync.dma_start(adste_sb[:], adsteT[:])

            # ---- pre-pass (layer 1): expand adst to per-edge values
            # (PSUM-resident), staged so the dlr broadcast has a full
            # iteration to land.  Layer 2 receives its per-edge adst
            # precomputed by layer 1 (same masks, fused projection). ----
            adst_ps = [None] * NB
            dlr_sb = [None] * NB
            mT_sb = [None] * NB
            obat = [None]

            def dlr_issue(b):
                if layer != 1:
                    return
                T = Tm[b]
                dlr = ppool.tile([P, T * P], dt.bfloat16, tag="dlr")
                if (PBM and b % PBM) or (PBMI and b % PBMI == 0):
                    dlrow = ppool.tile([1, T * P], dt.bfloat16, tag="dlrow")
                    nc.sync.dma_start(dlrow[:], dstlR[b:b + 1, 0:T * P])
                    half = (T // 2) * P
                    nc.gpsimd.partition_broadcast(dlr[:, 0:half],
                                                  dlrow[:, 0:half])
                    nc.gpsimd.partition_broadcast(dlr[:, half:T * P],
                                                  dlrow[:, half:T * P])
                else:
                    nc.sync.dma_start(
                        dlr[:],
                        dstlR[b:b + 1, 0:T * P].broadcast_to([P, T * P]))
                dlr_sb[b] = dlr

            MTE = os.environ.get("GAT_MTE", "AAD")  # mT copy engines

            def prepass_block(b):
                if layer != 1:
                    return
                T = Tm[b]
                # build mT = per-tile PE transpose of the mk masks (PE is
                # idle; avoids the dst-local replication broadcast entirely)
                mT = ppool.tile([P, T, P], dt.bfloat16, tag="mT", bufs=LKP + 2)
                mk = mk_sb[b]
                for ci, c0 in enumerate(range(0, T, 8)):
                    cn = min(8, T - c0)
                    mtt = psA.tile([P, 8, P], dt.bfloat16, tag="mtt")
                    for t in range(cn):
                        nc.tensor.transpose(mtt[:, t, :], mk[:, c0 + t, :],
                                            id_sb[:])
                    eng = MTE[(b + ci) % len(MTE)]
                    if eng == "A":
                        nc.scalar.activation(
                            mT[:, c0:c0 + cn, :], mtt[:, 0:cn, :],
                            mybir.ActivationFunctionType.Copy)
                    elif eng == "P":
                        nc.gpsimd.tensor_copy(mT[:, c0:c0 + cn, :],
                                              mtt[:, 0:cn, :])
                    else:
                        nc.vector.tensor_copy(mT[:, c0:c0 + cn, :],
                                              mtt[:, 0:cn, :])
                ap_ps = psD.tile([P, T * H], dt.float32, tag="adps")
                for t in range(T):
                    nc.tensor.matmul(ap_ps[:, t * H:(t + 1) * H],
                                     lhsT=mT[:, t, :],
                                     rhs=adst_sh[:, b * H:(b + 1) * H],
                                     start=True, stop=True)
                adst_ps[b] = ap_ps
                mT_sb[b] = mT

            # ---- gathers: B first (covers tail incl. boundary dummies),
            # then A overwrites its region ----
            htabA = htab[0:SPLIT, :]
            htabB = htab[SPLIT:NPAD, :]

            g_sb = [None] * NB
            mk_sb = [None] * NB

            def issue_gatherB(b):
                T = Tm[b]
                g = pb.tile([P, T, TABC], dt.bfloat16, tag="gath")
                sB = scol[b] + niA16[b] // 16
                if niB[b] > 0:
                    nc.gpsimd.dma_gather(
                        g[:, fAm[b]:T, :], htabB,
                        idx_sb[:, sB:sB + niB[b] // 16],
                        niB[b], niB[b], TABC, single_packet=SP)
                g_sb[b] = g

            def issue_gatherA(b):
                T = Tm[b]
                g = g_sb[b]
                sA = scol[b]
                if niA16[b] > 0:
                    a_tiles = -(-niA16[b] // P)
                    nc.gpsimd.dma_gather(
                        g[:, 0:a_tiles, :], htabA,
                        idx_sb[:, sA:sA + niA16[b] // 16],
                        niA16[b], niA16[b], TABC, single_packet=SP)

            def build_masks(b):
                T = Tm[b]
                # dst one-hot masks (pair-packed for DVE 2x)
                dl2 = sm.tile([P, T, 2], dt.bfloat16, tag="dl2")
                nc.scalar.activation(
                    dl2[:],
                    dstl_sb[:, toff[b]:toff[b] + T].rearrange(
                        "p (t o) -> p t o", o=1).broadcast_to([P, T, 2]),
                    mybir.ActivationFunctionType.Copy)
                mk = sm.tile([P, T, P], dt.bfloat16, tag="mk")   # [e_p,(t,d)]
                nc.vector.tensor_tensor(
                    mk[:].rearrange("p t (d2 pr) -> p t d2 pr", pr=2),
                    ior_sb[:].rearrange("p (t d2 pr) -> p t d2 pr", t=1, pr=2
                                        ).broadcast_to([P, T, P // 2, 2]),
                    dl2[:].rearrange("p t (d2 pr) -> p t d2 pr", d2=1
                                     ).broadcast_to([P, T, P // 2, 2]),
                    mybir.AluOpType.is_equal)
                mk_sb[b] = mk

            def epilogue(b, agg):
                # out = num/(den+eps) (+bias) (+ELU and fused proj, layer 1)
                dn = sm.tile([P, H], dt.float32, tag="dn")
                nc.vector.tensor_scalar_add(dn[:], agg[:, COUT:AGC], 1e-16)
                rc = sm.tile([P, H], dt.float32, tag="rc")
                nc.vector.reciprocal(rc[:], dn[:])
                if layer == 1:
                    ob = sm.tile([P, COUT], dt.bfloat16, tag="ob")
                    for h in range(H):
                        nc.scalar.activation(ob[:, h * F:(h + 1) * F],
                                             agg[:, h * F:(h + 1) * F],
                                             mybir.ActivationFunctionType.Copy,
                                             scale=rc[:, h:h + 1])
                    if not zero_bias:
                        nc.vector.tensor_add(
                            ob[:], ob[:],
                            b_sb[:].bitcast(dt.bfloat16)[:, 1::2])
                    # elu(y) = relu(y) + exp(min(y,0)) - 1
                    r1 = sm.tile([P, COUT], dt.bfloat16, tag="r1")
                    nc.scalar.activation(r1[:], ob[:],
                                         mybir.ActivationFunctionType.Relu,
                                         scale=-1.0)
                    r2 = sm.tile([P, COUT], dt.bfloat16, tag="r2")
                    nc.scalar.activation(r2[:], r1[:],
                                         mybir.ActivationFunctionType.Exp,
                                         scale=-1.0)
                    nc.scalar.activation(ob[:], ob[:],
                                         mybir.ActivationFunctionType.Relu)
                    nc.vector.scalar_tensor_tensor(
                        ob[:], r2[:], -1.0, ob[:],
                        mybir.AluOpType.add, mybir.AluOpType.add)
                    # fused layer-2 row production:
                    # [elu(out1) @ [W2|wasrc2|wadst2]] -> [h2|asrc2|adst2]
                    ps_t = psA.tile([P, 2, P], dt.bfloat16, tag="pst")
                    for c in range(2):
                        nc.tensor.transpose(ps_t[:, c, :],
                                            ob[:, c * P:(c + 1) * P],
                                            id_sb[:])
                    x2T = sm.tile([P, 2, P], dt.bfloat16, tag="x2T")
                    nc.scalar.activation(x2T[:], ps_t[:],
                                         mybir.ActivationFunctionType.Copy)
                    ps2 = psA.tile([P, 128], dt.float32, tag="ps2")
                    for c in range(2):
                        nc.tensor.matmul(ps2[:, 0:66], lhsT=x2T[:, c, :],
                                         rhs=w2_sb[:, c, :],
                                         start=(c == 0), stop=(c == 1))
                    b0 = (b // 8) * 8
                    if b == b0:
                        h8 = sm.tile([P, 8, 68], dt.bfloat16, tag="hst2")
                        obat[0] = h8
                    hst2 = obat[0]
                    nc.scalar.activation(hst2[:, b - b0, 0:64], ps2[:, 0:64],
                                         mybir.ActivationFunctionType.Copy)
                    nc.vector.tensor_copy(
                        hst2[:, b - b0, 64:68].bitcast(dt.float32),
                        ps2[:, 64:66])
                    if b == min(b0 + 7, NB - 1):
                        bn = b - b0 + 1
                        nc.sync.dma_start(
                            outT2[b0 * P:(b0 + bn) * P, :].rearrange(
                                "(t n) c -> n t c", t=bn),
                            hst2[:, 0:bn, :])
                    # layer-2 per-edge adst via the still-resident mT masks
                    a2b = sm.tile([P, 1], dt.bfloat16, tag="a2b")
                    nc.scalar.activation(a2b[:], ps2[:, 65:66],
                                         mybir.ActivationFunctionType.Copy)
                    T_ = Tm[b]
                    for t in range(T_):
                        nc.tensor.matmul(ps2[:, 66 + t:67 + t],
                                         lhsT=mT_sb[b][:, t, :],
                                         rhs=a2b[:],
                                         start=True, stop=True)
                    mT_sb[b] = None
                    nc.vector.tensor_copy(
                        a2all[:, toff[b]:toff[b] + T_], ps2[:, 66:66 + T_])
                else:
                    b0 = (b // 8) * 8
                    if b == b0:
                        ob8 = sm.tile([P, 8, COUT], dt.float32, tag="ob")
                        obat[0] = ob8
                    nc.scalar.activation(obat[0][:, b - b0, :], agg[:, 0:COUT],
                                         mybir.ActivationFunctionType.Copy,
                                         scale=rc[:, 0:1])
                    if not zero_bias:
                        nc.vector.tensor_add(obat[0][:, b - b0, :],
                                             obat[0][:, b - b0, :], b_sb[:])
                    if b == min(b0 + 7, NB - 1):
                        bn = b - b0 + 1
                        nc.sync.dma_start(
                            outT[b0 * P:(b0 + bn) * P, :].rearrange(
                                "(t n) c -> n t c", t=bn),
                            obat[0][:, 0:bn, :])

            # ---- phase B: software-pipelined per-block message passing.
            # Emission order is tuned for the in-order engine queues: the
            # et->prelu->exp->hp critical chain leads, lookahead issues fill
            # the ACT round-trip, the lagged epilogue never blocks it. ----
            LKP = min(PPK - 1, GLA)        # prepass needs mk already built
            for q in range(min(GLB, NB)):
                issue_gatherB(q)
            for q in range(min(GLA, NB)):
                issue_gatherA(q)
                build_masks(q)
            for q in range(min(LKP, NB)):
                prepass_block(q)
            pend = None                     # (block, agg) awaiting epilogue
            for b in range(NB):
                T = Tm[b]
                g = g_sb[b]
                mk = mk_sb[b]

                # w2 = exp(prelu(asrc + adst)) pair-broadcast, on ACT
                et = sm.tile([P, T * H], dt.float32, tag="et")
                if layer == 1:
                    adst_in = adst_ps[b][:].rearrange("p (t h) -> p t h", h=H)
                else:
                    adst_in = adste_sb[:, toff[b]:toff[b] + T].rearrange(
                        "p (t h) -> p t h", h=H)
                nc.vector.tensor_tensor(
                    et[:].rearrange("p (t h) -> p t h", h=H),
                    g[:, :, COUT:COUT + 2 * H].bitcast(dt.float32),
                    adst_in,
                    mybir.AluOpType.add)
                adst_ps[b] = None
                lr = sm.tile([P, T * H], dt.float32, tag="lr")
                nc.scalar.activation(lr[:], et[:],
                                     mybir.ActivationFunctionType.Prelu,
                                     alpha=NEG)
                wt2 = sm.tile([P, T, H, 2], dt.bfloat16, tag="wt2")
                nc.scalar.activation(
                    wt2[:],
                    lr[:].rearrange("p (t h o) -> p t h o", h=H, o=1
                                    ).broadcast_to([P, T, H, 2]),
                    mybir.ActivationFunctionType.Exp)

                # lookahead issues (fill the ACT round-trip gap on DVE/Pool;
                # the dlr replication goes last so it never delays a gather)
                if not ALATE and b + GLA < NB:
                    issue_gatherA(b + GLA)
                if b + GLB < NB:
                    issue_gatherB(b + GLB)
                if pend is not None and not EPL:
                    epilogue(*pend)
                    pend = None
                if b + GLA < NB:
                    if ALATE:
                        issue_gatherA(b + GLA)
                    build_masks(b + GLA)
                if b + LKP < NB:
                    prepass_block(b + LKP)

                # hp = [w * h | w]  (pair-packed 2x multiply, split in halves
                # so aggregation can start on the first half early)
                hp = sm.tile([P, T, AGC], dt.bfloat16, tag="hp")
                agg = psB.tile([P, AGC], dt.float32, tag="agg")
                nhs = min(HPS, T)
                bnds = [t * T // nhs for t in range(nhs)] + [T]
                for t0, t1 in zip(bnds[:-1], bnds[1:]):
                    if t0 >= t1:
                        continue
                    ts = t1 - t0
                    nc.vector.tensor_tensor(
                        hp[:, t0:t1, 0:COUT].rearrange(
                            "p t (h f2 pr) -> p t h f2 pr", h=H, pr=2),
                        g[:, t0:t1, 0:COUT].rearrange(
                            "p t (h f2 pr) -> p t h f2 pr", h=H, pr=2),
                        wt2[:, t0:t1].rearrange(
                            "p t (h1 h) pr -> p t h h1 pr", h1=1
                        ).broadcast_to([P, ts, H, F // 2, 2]),
                        mybir.AluOpType.mult)
                    nc.scalar.activation(
                        hp[:, t0:t1, COUT:AGC],
                        wt2[:, t0:t1, :, 0],
                        mybir.ActivationFunctionType.Copy)
                    for t in range(t0, t1):
                        nc.tensor.matmul(agg[:], lhsT=mk[:, t, :],
                                         rhs=hp[:, t, :],
                                         start=(t == 0), stop=(t == T - 1))
                g_sb[b] = None
                mk_sb[b] = None
                if pend is not None and EPL:
                    epilogue(*pend)
                pend = (b, agg)
            epilogue(*pend)
            if layer == 1:
                nc.sync.dma_start(outA2[:], a2all[:])

    nc.compile()
    return nc


# ----------------------------------------------------------------------------
# host-side weight packing
# ----------------------------------------------------------------------------

def _expand_att(att, H, F):
    out = np.zeros((H * F, H), np.float32)
    for h in range(H):
        out[h * F:(h + 1) * F, h] = att[h]
    return out


def _inputs_layer(meta, idx_all, dstl_pt, dstl_row, b, layer):
    H = H1 if layer == 1 else 1
    COUT = H * F1
    b_np = np.asarray(b, np.float32).reshape(1, COUT)
    ior = np.arange(P, dtype=np.float32).reshape(1, P).astype(BF16)
    ioc = np.arange(P, dtype=np.float32).reshape(P, 1)
    in_maps = []
    for m in range(NCORES):
        entry = {
            "idxT": idx_all[m],
            "dstlT": dstl_pt[m].astype(BF16),
            "brow": b_np, "iot_r": ior, "iot_c": ioc,
        }
        if layer == 1:
            entry["dstlR"] = dstl_row[m]
        in_maps.append(entry)
    return in_maps


# ----------------------------------------------------------------------------
# entry point
# ----------------------------------------------------------------------------

def kernel(x, edge_index, W1, att_src1, att_dst1, b1, W2, att_src2, att_dst2,
           b2):
    x = np.asarray(x, np.float32)
    edge_index = np.asarray(edge_index)

    meta, idx_all, dstl_pt, dstl_row = _prep_edges(edge_index)
    asg = np.asarray(meta["asg"])                     # [NB, NCORES]

    # ---- launch 0: per-node projection [h | asrc | adst] ----
    key0 = (0,)
    if key0 not in _prog_cache:
        _prog_cache[key0] = _build_proj()
    nc0 = _prog_cache[key0]

    W1f = np.asarray(W1, np.float32)
    wasrc1 = W1f @ _expand_att(np.asarray(att_src1, np.float32), H1, F1)
    wadst1 = W1f @ _expand_att(np.asarray(att_dst1, np.float32), H1, F1)
    wr_np = np.concatenate([W1f, wasrc1, wadst1], axis=1)
    wr_np = np.ascontiguousarray(wr_np.reshape(1, P, 256 + 2 * H1)).astype(BF16)

    xpad = np.zeros((NCORES * SHARD, IN), np.float32)
    xpad[:N] = x
    in_maps0 = []
    for m in range(NCORES):
        shard = xpad[m * SHARD:(m + 1) * SHARD]
        xs_np = np.ascontiguousarray(
            shard.reshape(NB, P, 1, P).transpose(2, 3, 0, 1)).astype(BF16)
        in_maps0.append({"xs": xs_np, "wr": wr_np})
    res0 = run_bass_kernel_spmd(nc0, in_maps0, list(range(NCORES))).results

    COUT1 = H1 * F1
    htab1 = np.zeros((NPAD, 384), BF16)
    adst1 = np.zeros((N + P, H1), np.float32)         # per-node adst (layer 1)
    for m in range(NCORES):
        lo = m * SHARD
        hi = min(NPAD, (m + 1) * SHARD)
        hts = res0[m]["hts"][:hi - lo]
        htab1[lo:hi, 0:COUT1 + 2 * H1] = hts[:, 0:COUT1 + 2 * H1]
        adst1[lo:hi] = hts[:, COUT1 + 2 * H1:].copy().view(np.float32)

    # per-core adst in assigned-block order: [P, NB*H] (partition = dst local)
    def adst_input(adst_n, H):
        out = []
        for m in range(NCORES):
            a = np.zeros((NB, P, H), np.float32)
            for i in range(NB):
                g = asg[i, m]
                rows = adst_n[g * P:(g + 1) * P]
                a[i, :len(rows)] = rows
            out.append(np.ascontiguousarray(
                a.transpose(1, 0, 2).reshape(P, NB * H)).astype(BF16))
        return out

    # ---- layer 1 ----
    tkey = tuple(meta["Tm"])
    zb1 = bool(np.all(np.asarray(b1) == 0))
    key1 = (1, tkey, zb1)
    if key1 not in _prog_cache:
        _prog_cache[key1] = _build_layer(meta, 1, zb1)
    ncA = _prog_cache[key1]
    in_maps = _inputs_layer(meta, idx_all, dstl_pt, dstl_row, b1, 1)
    W2f = np.asarray(W2, np.float32)
    wasrc2 = W2f @ np.asarray(att_src2, np.float32).reshape(F1, 1)
    wadst2 = W2f @ np.asarray(att_dst2, np.float32).reshape(F1, 1)
    w2r_np = np.ascontiguousarray(
        np.concatenate([W2f, wasrc2, wadst2], axis=1).reshape(2, P, 66)
    ).astype(BF16)
    ident = np.eye(P, dtype=np.float32).astype(BF16)
    a1in = adst_input(adst1, H1)
    for m, mmap in enumerate(in_maps):
        mmap["w2r"] = w2r_np
        mmap["identT"] = ident
        mmap["htab"] = htab1
        mmap["adstT"] = a1in[m]
    resA = run_bass_kernel_spmd(ncA, in_maps, list(range(NCORES))).results

    # reassemble layer-2 table from assigned-block outputs
    htab2 = np.zeros((NPAD, 128), BF16)
    for m in range(NCORES):
        o2 = resA[m]["outT2"]
        for i in range(NB):
            g = asg[i, m]
            if g * P >= NPAD:
                continue
            hi = min(NPAD, (g + 1) * P) - g * P
            htab2[g * P:g * P + hi, 0:66] = o2[i * P:i * P + hi, 0:66]

    # ---- layer 2 ----
    zb2 = bool(np.all(np.asarray(b2) == 0))
    key2 = (2, tkey, zb2)
    if key2 not in _prog_cache:
        _prog_cache[key2] = _build_layer(meta, 2, zb2)
    ncB = _prog_cache[key2]
    in_maps2 = _inputs_layer(meta, idx_all, dstl_pt, dstl_row, b2, 2)
    for m, mmap in enumerate(in_maps2):
        mmap["htab"] = htab2
        mmap["adsteT"] = resA[m]["outA2"]
    resB = run_bass_kernel_spmd(ncB, in_maps2, list(range(NCORES))).results

    out = np.zeros((N, F1), np.float32)
    for m in range(NCORES):
        o = resB[m]["outT"]
        for i in range(NB):
            g = asg[i, m]
            lo = g * P
            if lo >= N:
                continue
            hi = min(N, lo + P)
            out[lo:hi] = o[i * P:i * P + (hi - lo)].astype(np.float32)
    return out


# revision 52
# speedup vs baseline: 1.0220x; 1.0220x over previous
"""2-layer GAT on 8 Trainium2 NeuronCores (Bass/Tile).

Sharding: the 391 dst 128-node blocks are sorted by half-A edge count and
dealt in groups of 8 to the cores (one block per core per iteration), so the
per-iteration cross-core tile maximum stays near the mean.  Edges are routed
to the core owning their dst block; each block's edges occupy disjoint
half-A / half-B tile ranges (table halves keep dma_gather indices within
int16), so the two gathers per block never overlap and pipeline freely.

Per-layer device program (phase B only; projections are fused elsewhere):
  per 128-dst block: dma_gather A + B from the HBM row table
  [h | asrc f32-bits], one-hot dst masks on DVE (pair-packed 2x),
  w = exp(prelu(asrc+adst)) on ACT, weighted rows on DVE, aggregation +
  softmax denominator via PSUM-accumulated matmuls, epilogue scaling on ACT.

Per-edge adst: layer 1 expands its own via maskT matmuls (PSUM-resident);
the transposed masks come from PE-transposing the mk masks (no dst-local
replication broadcast), and the same masks also expand layer 2's adst (from
the fused [h2|asrc2|adst2] projection), written to HBM so layer 2 runs with
no pre-pass at all.

Launch 0 projects [h | asrc | adst] per node; layer 1's epilogue fuses the
layer-2 projection, so neither layer loads x.  Shards are exchanged through
the host between launches.
"""

import os
import numpy as np
import ml_dtypes

import concourse.bass as bass
import concourse.bacc as bacc
import concourse.tile as tile
from concourse import mybir
from concourse.bass_utils import run_bass_kernel_spmd

BF16 = ml_dtypes.bfloat16

N = 50000
E = 800000
IN = 128
H1 = 4
F1 = 64
NEG = 0.2
P = 128
NCORES = 8
NB = 49                 # block iterations per core
SHARD = NB * P          # 6272 rows per core in the table
NPAD = 391 * P          # 50048 padded node count
NGB = NCORES * NB       # 392 block slots (391 real + 1 dummy)
SPLIT = 196 * P         # 25088: gather-table half boundary (int16 idx limit)
GRP = 16                # proj-launch load group

_prog_cache = {}


# ----------------------------------------------------------------------------
# host-side edge preprocessing (shared by both layers)
# ----------------------------------------------------------------------------

def _prep_edges(edge_index):
    src = np.concatenate([edge_index[0].astype(np.int64), np.arange(N, dtype=np.int64)])
    dst = np.concatenate([edge_index[1].astype(np.int64), np.arange(N, dtype=np.int64)])
    order = np.argsort(dst, kind="stable")
    s = src[order]
    d = dst[order]

    gb = d >> 7                                   # global 128-block of dst
    cnt = np.bincount(gb, minlength=NGB)
    starts = np.concatenate([[0], np.cumsum(cnt)])
    isB = s >= SPLIT
    cntA = np.zeros(NGB, np.int64)
    for g in range(NGB):
        cntA[g] = np.count_nonzero(~isB[starts[g]:starts[g + 1]])

    # deal blocks sorted by half-A count: iteration i gets ranks [8i, 8i+8),
    # then anneal block swaps between iterations to shrink the per-iteration
    # cross-core tile maxima (the SPMD padding)
    blk_order = np.argsort(-cntA, kind="stable")
    asg = blk_order.reshape(NB, NCORES).copy()    # [iter, core] -> global block
    cntBg = cnt - cntA

    def _iter_cost(rows):
        a = cntA[rows].max()
        b = cntBg[rows].max()
        ta = -(-a // P)
        return ta + max(-(-b // P), 1 - ta)

    icost = np.array([_iter_cost(asg[i]) for i in range(NB)], np.int64)
    rng = np.random.default_rng(7)
    ii = rng.integers(0, NB, 400000)
    jj = rng.integers(0, NB, 400000)
    mm = rng.integers(0, NCORES, 400000)
    nn = rng.integers(0, NCORES, 400000)
    for k in range(400000):
        i, j, m, n = ii[k], jj[k], mm[k], nn[k]
        if i == j:
            continue
        asg[i, m], asg[j, n] = asg[j, n], asg[i, m]
        ci, cj = _iter_cost(asg[i]), _iter_cost(asg[j])
        if ci + cj <= icost[i] + icost[j]:
            icost[i], icost[j] = ci, cj
        else:
            asg[i, m], asg[j, n] = asg[j, n], asg[i, m]

    nA = cntA[asg]                                # [NB, NCORES]
    nBc = cntBg[asg]
    nAmax = nA.max(1)                             # [NB]
    nBmax = nBc.max(1)
    # disjoint A/B tile ranges: no write overlap between the two gathers,
    # so they pipeline freely (a shared boundary tile would serialize them)
    TA = -(-nAmax // P)
    TB = -(-nBmax // P)
    TB = np.maximum(TB, 1 - TA)                   # dummy block: >= 1 tile
    Tm = (TA + TB).astype(np.int64)
    niA16 = (TA * P).astype(np.int64)             # static gather-A num_idxs
    fA = TA                                       # A tile count
    rA = np.zeros(NB, np.int64)
    niB = (TB * P).astype(np.int64)               # static gather-B num_idxs

    toff = np.zeros(NB + 1, np.int64)
    np.cumsum(Tm, out=toff[1:])
    Ttot = int(toff[NB])
    TMX = int(Tm.max())

    scol = np.zeros(NB + 1, np.int64)             # idx column offsets (per 16)
    np.cumsum(niA16 // 16 + niB // 16, out=scol[1:])
    Stot = int(scol[NB])

    idx_all = np.zeros((NCORES, P, Stot), np.int16)
    dstl = np.full((NCORES, Ttot, P), -1.0, np.float32)   # [t, p] layout

    for i in range(NB):
        sA = int(scol[i])
        sB = sA + int(niA16[i] // 16)
        for m in range(NCORES):
            g = asg[i, m]
            e0, e1 = starts[g], starts[g + 1]
            sb = s[e0:e1]
            mB = isB[e0:e1]
            shA = sb[~mB]
            shB = sb[mB] - SPLIT
            dlA = (d[e0:e1][~mB] - (g << 7)).astype(np.float32)
            dlB = (d[e0:e1][mB] - (g << 7)).astype(np.float32)
            na, nb_ = len(shA), len(shB)
            # gather-A idx: real | dummy-0 pad to its tile range
            ia = np.zeros(int(niA16[i]), np.int16)
            ia[:na] = shA
            # gather-B idx: real | dummy-0 pad to its tile range
            ib = np.zeros(int(niB[i]), np.int16)
            ib[:nb_] = shB
            for seg, off in ((ia, sA), (ib, sB)):
                if len(seg) == 0:
                    continue
                w = seg.reshape(-1, 16).T          # [16, S]
                idx_all[m][:, off:off + w.shape[1]] = np.tile(w, (8, 1))
            dl = np.full(int(Tm[i]) * P, -1.0, np.float32)
            dl[:na] = dlA
            boff = int(niA16[i])
            dl[boff:boff + nb_] = dlB
            dstl[m][toff[i]:toff[i] + Tm[i], :] = dl.reshape(int(Tm[i]), P)

    dstl_pt = np.ascontiguousarray(dstl.transpose(0, 2, 1))   # [m, P, Ttot]
    dstl_row = np.full((NCORES, NB, TMX * P), -1.0, np.float32)
    for m in range(NCORES):
        for i in range(NB):
            T = int(Tm[i])
            dstl_row[m, i, :T * P] = dstl[m, toff[i]:toff[i] + T].reshape(-1)
    dstl_row = dstl_row.astype(BF16)

    meta = dict(Tm=Tm.tolist(), toff=toff.tolist(), fA=fA.tolist(),
                rA=rA.tolist(), niA16=niA16.tolist(), niB=niB.tolist(),
                scol=scol.tolist(), Ttot=Ttot, Stot=Stot, Tmax=TMX,
                asg=asg.tolist())
    return meta, idx_all, dstl_pt, dstl_row


# ----------------------------------------------------------------------------
# launch 0: project own shard -> [h | asrc | adst] table slice
# ----------------------------------------------------------------------------

def _build_proj():
    dt = mybir.dt
    GRPP = int(os.environ.get("GAT_GRP0", "8"))
    KCH, H = 1, H1
    COUT = H1 * F1
    RC = COUT + 2 * H                   # [h | asrc | adst]
    OCOL = COUT + 4 * H                 # bf16 slots: h | asrc bits | adst bits
    nc = bacc.Bacc("TRN2", target_bir_lowering=False, debug=False,
                   num_devices=NCORES)
    xs = nc.dram_tensor("xs", [KCH, P, NB, P], dt.bfloat16,
                        kind="ExternalInput")
    wr = nc.dram_tensor("wr", [KCH, P, RC], dt.bfloat16,
                        kind="ExternalInput")
    hts = nc.dram_tensor("hts", [SHARD, OCOL], dt.bfloat16,
                         kind="ExternalOutput")
    with tile.TileContext(nc) as tc:
        with (
            tc.tile_pool(name="const", bufs=1) as cp,
            tc.tile_pool(name="pa", bufs=3) as pa,
            tc.tile_pool(name="psA", bufs=3, space="PSUM") as psA,
        ):
            wr_sb = cp.tile([P, KCH, RC], dt.bfloat16)
            nc.sync.dma_start(wr_sb[:], wr[:].rearrange("k p c -> p k c"))
            for g0 in range(0, NB, GRPP):
                gn = min(GRPP, NB - g0)
                xa = pa.tile([P, KCH, gn, P], dt.bfloat16, tag="xa")
                nc.sync.dma_start(
                    xa[:], xs[:, :, g0:g0 + gn, :].rearrange(
                        "k f t n -> f k t n"))
                hst = pa.tile([P, gn, OCOL], dt.bfloat16, tag="hst")
                for t0 in range(0, gn, 2):
                    pn = min(2, gn - t0)
                    ps = psA.tile([P, 2, 512], dt.float32, tag="psa")
                    for t2 in range(pn):
                        for k in range(KCH):
                            nc.tensor.matmul(ps[:, t2, 0:RC],
                                             lhsT=xa[:, k, t0 + t2, :],
                                             rhs=wr_sb[:, k, 0:RC],
                                             start=(k == 0),
                                             stop=(k == KCH - 1))
                    nc.scalar.activation(
                        hst[:, t0:t0 + pn, 0:COUT], ps[:, 0:pn, 0:COUT],
                        mybir.ActivationFunctionType.Copy)
                    nc.vector.tensor_copy(
                        hst[:, t0:t0 + pn, COUT:OCOL].bitcast(dt.float32),
                        ps[:, 0:pn, COUT:COUT + 2 * H])
                nc.sync.dma_start(
                    hts[g0 * P:(g0 + gn) * P, :].rearrange(
                        "(t n) c -> n t c", t=gn),
                    hst[:])
    nc.compile()
    return nc


# ----------------------------------------------------------------------------
# per-layer message-passing program (phase B)
# ----------------------------------------------------------------------------

def _build_layer(meta, layer, zero_bias):
    """layer 1: heads 4, F 64, fused layer-2 row production, no dense out.
    layer 2: heads 1, F 64, out f32 [SHARD, 64]."""
    dt = mybir.dt
    Tm, toff, fAm = meta["Tm"], meta["toff"], meta["fA"]
    niA16, niB, scol = meta["niA16"], meta["niB"], meta["scol"]
    Ttot, Stot, Tmax = meta["Ttot"], meta["Stot"], meta["Tmax"]

    if layer == 1:
        H, F = H1, F1
    else:
        H, F = 1, F1
    COUT = H * F
    AGC = COUT + H                    # aggregation psum cols: [num | den]
    TABC = 384 if layer == 1 else 128  # table row slots (256B granules)

    nc = bacc.Bacc("TRN2", target_bir_lowering=False, debug=False,
                   num_devices=NCORES)

    if layer == 1:
        w2r = nc.dram_tensor("w2r", [2, P, 66], dt.bfloat16,
                             kind="ExternalInput")
        identT = nc.dram_tensor("identT", [P, P], dt.bfloat16,
                                kind="ExternalInput")
        outT2 = nc.dram_tensor("outT2", [SHARD, 68], dt.bfloat16,
                               kind="ExternalOutput")
        # per-edge layer-2 adst, computed here by reusing the mT masks
        outA2 = nc.dram_tensor("outA2", [P, Ttot], dt.float32,
                               kind="ExternalOutput")
        dstlR = nc.dram_tensor("dstlR", [NB, Tmax * P], dt.bfloat16,
                               kind="ExternalInput")
        adstT = nc.dram_tensor("adstT", [P, NB * H], dt.bfloat16,
                               kind="ExternalInput")
    else:
        outT = nc.dram_tensor("outT", [SHARD, COUT], dt.float32,
                              kind="ExternalOutput")
        adsteT = nc.dram_tensor("adsteT", [P, Ttot], dt.float32,
                                kind="ExternalInput")
    idxT = nc.dram_tensor("idxT", [P, Stot], dt.int16, kind="ExternalInput")
    dstlT = nc.dram_tensor("dstlT", [P, Ttot], dt.bfloat16, kind="ExternalInput")
    brow = nc.dram_tensor("brow", [1, COUT], dt.float32, kind="ExternalInput")
    iot_r = nc.dram_tensor("iot_r", [1, P], dt.bfloat16, kind="ExternalInput")
    iot_c = nc.dram_tensor("iot_c", [P, 1], dt.float32, kind="ExternalInput")
    htab = nc.dram_tensor("htab", [NPAD, TABC], dt.bfloat16,
                          kind="ExternalInput")

    SP = bool(int(os.environ.get("GAT_SP", "0")))
    PBB = int(os.environ.get("GAT_PBB", "5"))
    PPK = int(os.environ.get("GAT_PPK", "3"))
    PBM = int(os.environ.get("GAT_PB_MOD%d" % layer,
                             "0" if layer == 1 else "2"))
    PBMI = int(os.environ.get("GAT_PBI%d" % layer, "0"))  # 1/K blocks on Pool
    GLB = int(os.environ.get("GAT_GLB", "3" if layer == 1 else "2"))
    GLA = int(os.environ.get("GAT_GLA", "2" if layer == 1 else "1"))
    SMB = int(os.environ.get("GAT_SMB", "4"))
    ALATE = bool(int(os.environ.get("GAT_ALATE", "1")))
    HPS = int(os.environ.get("GAT_HPS", "2"))      # hp split count
    EPL = bool(int(os.environ.get("GAT_EPL%d" % layer, "0")))  # epilogue after hp

    with tile.TileContext(nc) as tc:
        with (
            tc.tile_pool(name="const", bufs=1) as cp,
            tc.tile_pool(name="keep", bufs=1) as kp,
            tc.tile_pool(name="pp", bufs=3) as ppool,
            tc.tile_pool(name="pb", bufs=PBB) as pb,
            tc.tile_pool(name="sm", bufs=SMB) as sm,
            tc.tile_pool(name="psA", bufs=1, space="PSUM") as psA,
            tc.tile_pool(name="psB", bufs=2, space="PSUM") as psB,
            tc.tile_pool(name="psD", bufs=min(PPK - 1, GLA) + 1,
                         space="PSUM") as psD,
        ):
            # ---- resident constants ----
            b_sb = cp.tile([P, COUT], dt.float32)
            nc.sync.dma_start(b_sb[:], brow[:].broadcast_to([P, COUT]))
            ior_sb = cp.tile([P, P], dt.bfloat16)
            nc.sync.dma_start(ior_sb[:], iot_r[:].broadcast_to([P, P]))
            ioc_sb = cp.tile([P, 1], dt.float32)
            nc.sync.dma_start(ioc_sb[:], iot_c[:])
            if layer == 1:
                w2_sb = cp.tile([P, 2, 66], dt.bfloat16)
                nc.sync.dma_start(w2_sb[:], w2r[:].rearrange("k p c -> p k c"))
                id_sb = cp.tile([P, P], dt.bfloat16)
                nc.sync.dma_start(id_sb[:], identT[:])
            idx_sb = kp.tile([P, Stot], dt.int16)
            nc.sync.dma_start(idx_sb[:], idxT[:])
            dstl_sb = kp.tile([P, Ttot], dt.bfloat16)
            nc.sync.dma_start(dstl_sb[:], dstlT[:])
            if layer == 1:
                adst_sh = kp.tile([P, NB * H], dt.bfloat16)
                nc.sync.dma_start(adst_sh[:], adstT[:])
                a2all = kp.tile([P, Ttot], dt.float32)
            else:
                adste_sb = kp.tile([P, Ttot], dt.float32)
                nc.sync.dma_start(adste_sb[:], adsteT[:])

            # ---- pre-pass (layer 1): expand adst to per-edge values
            # (PSUM-resident), staged so the dlr broadcast has a full
            # iteration to land.  Layer 2 receives its per-edge adst
            # precomputed by layer 1 (same masks, fused projection). ----
            adst_ps = [None] * NB
            dlr_sb = [None] * NB
            mT_sb = [None] * NB
            obat = [None]

            def dlr_issue(b):
                if layer != 1:
                    return
                T = Tm[b]
                dlr = ppool.tile([P, T * P], dt.bfloat16, tag="dlr")
                if (PBM and b % PBM) or (PBMI and b % PBMI == 0):
                    dlrow = ppool.tile([1, T * P], dt.bfloat16, tag="dlrow")
                    nc.sync.dma_start(dlrow[:], dstlR[b:b + 1, 0:T * P])
                    half = (T // 2) * P
                    nc.gpsimd.partition_broadcast(dlr[:, 0:half],
                                                  dlrow[:, 0:half])
                    nc.gpsimd.partition_broadcast(dlr[:, half:T * P],
                                                  dlrow[:, half:T * P])
                else:
                    nc.sync.dma_start(
                        dlr[:],
                        dstlR[b:b + 1, 0:T * P].broadcast_to([P, T * P]))
                dlr_sb[b] = dlr

            MTE = os.environ.get("GAT_MTE", "AAD")  # mT copy engines

            def prepass_block(b):
                if layer != 1:
                    return
                T = Tm[b]
                # build mT = per-tile PE transpose of the mk masks (PE is
                # idle; avoids the dst-local replication broadcast entirely)
                mT = ppool.tile([P, T, P], dt.bfloat16, tag="mT", bufs=LKP + 2)
                mk = mk_sb[b]
                for ci, c0 in enumerate(range(0, T, 8)):
                    cn = min(8, T - c0)
                    mtt = psA.tile([P, 8, P], dt.bfloat16, tag="mtt")
                    for t in range(cn):
                        nc.tensor.transpose(mtt[:, t, :], mk[:, c0 + t, :],
                                            id_sb[:])
                    eng = MTE[(b + ci) % len(MTE)]
                    if eng == "A":
                        nc.scalar.activation(
                            mT[:, c0:c0 + cn, :], mtt[:, 0:cn, :],
                            mybir.ActivationFunctionType.Copy)
                    elif eng == "P":
                        nc.gpsimd.tensor_copy(mT[:, c0:c0 + cn, :],
                                              mtt[:, 0:cn, :])
                    else:
                        nc.vector.tensor_copy(mT[:, c0:c0 + cn, :],
                                              mtt[:, 0:cn, :])
                ap_ps = psD.tile([P, T * H], dt.float32, tag="adps")
                for t in range(T):
                    nc.tensor.matmul(ap_ps[:, t * H:(t + 1) * H],
                                     lhsT=mT[:, t, :],
                                     rhs=adst_sh[:, b * H:(b + 1) * H],
                                     start=True, stop=True)
                adst_ps[b] = ap_ps
                mT_sb[b] = mT

            # ---- gathers: B first (covers tail incl. boundary dummies),
            # then A overwrites its region ----
            htabA = htab[0:SPLIT, :]
            htabB = htab[SPLIT:NPAD, :]

            g_sb = [None] * NB
            mk_sb = [None] * NB

            def issue_gatherB(b):
                T = Tm[b]
                g = pb.tile([P, T, TABC], dt.bfloat16, tag="gath")
                sB = scol[b] + niA16[b] // 16
                if niB[b] > 0:
                    nc.gpsimd.dma_gather(
                        g[:, fAm[b]:T, :], htabB,
                        idx_sb[:, sB:sB + niB[b] // 16],
                        niB[b], niB[b], TABC, single_packet=SP)
                g_sb[b] = g

            def issue_gatherA(b):
                T = Tm[b]
                g = g_sb[b]
                sA = scol[b]
                if niA16[b] > 0:
                    a_tiles = -(-niA16[b] // P)
                    nc.gpsimd.dma_gather(
                        g[:, 0:a_tiles, :], htabA,
                        idx_sb[:, sA:sA + niA16[b] // 16],
                        niA16[b], niA16[b], TABC, single_packet=SP)

            def build_masks(b):
                T = Tm[b]
                # dst one-hot masks (pair-packed for DVE 2x)
                dl2 = sm.tile([P, T, 2], dt.bfloat16, tag="dl2")
                nc.scalar.activation(
                    dl2[:],
                    dstl_sb[:, toff[b]:toff[b] + T].rearrange(
                        "p (t o) -> p t o", o=1).broadcast_to([P, T, 2]),
                    mybir.ActivationFunctionType.Copy)
                mk = sm.tile([P, T, P], dt.bfloat16, tag="mk")   # [e_p,(t,d)]
                nc.vector.tensor_tensor(
                    mk[:].rearrange("p t (d2 pr) -> p t d2 pr", pr=2),
                    ior_sb[:].rearrange("p (t d2 pr) -> p t d2 pr", t=1, pr=2
                                        ).broadcast_to([P, T, P // 2, 2]),
                    dl2[:].rearrange("p t (d2 pr) -> p t d2 pr", d2=1
                                     ).broadcast_to([P, T, P // 2, 2]),
                    mybir.AluOpType.is_equal)
                mk_sb[b] = mk

            def epilogue(b, agg):
                # out = num/(den+eps) (+bias) (+ELU and fused proj, layer 1)
                dn = sm.tile([P, H], dt.float32, tag="dn")
                nc.vector.tensor_scalar_add(dn[:], agg[:, COUT:AGC], 1e-16)
                rc = sm.tile([P, H], dt.float32, tag="rc")
                nc.vector.reciprocal(rc[:], dn[:])
                if layer == 1:
                    ob = sm.tile([P, COUT], dt.bfloat16, tag="ob")
                    for h in range(H):
                        nc.scalar.activation(ob[:, h * F:(h + 1) * F],
                                             agg[:, h * F:(h + 1) * F],
                                             mybir.ActivationFunctionType.Copy,
                                             scale=rc[:, h:h + 1])
                    if not zero_bias:
                        nc.vector.tensor_add(
                            ob[:], ob[:],
                            b_sb[:].bitcast(dt.bfloat16)[:, 1::2])
                    # elu(y) = relu(y) + exp(min(y,0)) - 1
                    r1 = sm.tile([P, COUT], dt.bfloat16, tag="r1")
                    nc.scalar.activation(r1[:], ob[:],
                                         mybir.ActivationFunctionType.Relu,
                                         scale=-1.0)
                    r2 = sm.tile([P, COUT], dt.bfloat16, tag="r2")
                    nc.scalar.activation(r2[:], r1[:],
                                         mybir.ActivationFunctionType.Exp,
                                         scale=-1.0)
                    nc.scalar.activation(ob[:], ob[:],
                                         mybir.ActivationFunctionType.Relu)
                    nc.vector.scalar_tensor_tensor(
                        ob[:], r2[:], -1.0, ob[:],
                        mybir.AluOpType.add, mybir.AluOpType.add)
                    # fused layer-2 row production:
                    # [elu(out1) @ [W2|wasrc2|wadst2]] -> [h2|asrc2|adst2]
                    ps_t = psA.tile([P, 2, P], dt.bfloat16, tag="pst")
                    for c in range(2):
                        nc.tensor.transpose(ps_t[:, c, :],
                                            ob[:, c * P:(c + 1) * P],
                                            id_sb[:])
                    x2T = sm.tile([P, 2, P], dt.bfloat16, tag="x2T")
                    nc.scalar.activation(x2T[:], ps_t[:],
                                         mybir.ActivationFunctionType.Copy)
                    ps2 = psA.tile([P, 128], dt.float32, tag="ps2")
                    for c in range(2):
                        nc.tensor.matmul(ps2[:, 0:66], lhsT=x2T[:, c, :],
                                         rhs=w2_sb[:, c, :],
                                         start=(c == 0), stop=(c == 1))
                    b0 = (b // 8) * 8
                    if b == b0:
                        h8 = sm.tile([P, 8, 68], dt.bfloat16, tag="hst2")
                        obat[0] = h8
                    hst2 = obat[0]
                    nc.scalar.activation(hst2[:, b - b0, 0:64], ps2[:, 0:64],
                                         mybir.ActivationFunctionType.Copy)
                    nc.vector.tensor_copy(
                        hst2[:, b - b0, 64:68].bitcast(dt.float32),
                        ps2[:, 64:66])
                    if b == min(b0 + 7, NB - 1):
                        bn = b - b0 + 1
                        nc.sync.dma_start(
                            outT2[b0 * P:(b0 + bn) * P, :].rearrange(
                                "(t n) c -> n t c", t=bn),
                            hst2[:, 0:bn, :])
                    # layer-2 per-edge adst via the still-resident mT masks
                    a2b = sm.tile([P, 1], dt.bfloat16, tag="a2b")
                    nc.scalar.activation(a2b[:], ps2[:, 65:66],
                                         mybir.ActivationFunctionType.Copy)
                    T_ = Tm[b]
                    for t in range(T_):
                        nc.tensor.matmul(ps2[:, 66 + t:67 + t],
                                         lhsT=mT_sb[b][:, t, :],
                                         rhs=a2b[:],
                                         start=True, stop=True)
                    mT_sb[b] = None
                    nc.vector.tensor_copy(
                        a2all[:, toff[b]:toff[b] + T_], ps2[:, 66:66 + T_])
                else:
                    b0 = (b // 8) * 8
                    if b == b0:
                        ob8 = sm.tile([P, 8, COUT], dt.float32, tag="ob")
                        obat[0] = ob8
                    nc.scalar.activation(obat[0][:, b - b0, :], agg[:, 0:COUT],
                                         mybir.ActivationFunctionType.Copy,
                                         scale=rc[:, 0:1])
                    if not zero_bias:
                        nc.vector.tensor_add(obat[0][:, b - b0, :],
                                             obat[0][:, b - b0, :], b_sb[:])
                    if b == min(b0 + 7, NB - 1):
                        bn = b - b0 + 1
                        nc.sync.dma_start(
                            outT[b0 * P:(b0 + bn) * P, :].rearrange(
                                "(t n) c -> n t c", t=bn),
                            obat[0][:, 0:bn, :])

            # ---- phase B: software-pipelined per-block message passing.
            # Emission order is tuned for the in-order engine queues: the
            # et->prelu->exp->hp critical chain leads, lookahead issues fill
            # the ACT round-trip, the lagged epilogue never blocks it. ----
            LKP = min(PPK - 1, GLA)        # prepass needs mk already built
            for q in range(min(GLB, NB)):
                issue_gatherB(q)
            for q in range(min(GLA, NB)):
                issue_gatherA(q)
                build_masks(q)
            for q in range(min(LKP, NB)):
                prepass_block(q)
            pend = None                     # (block, agg) awaiting epilogue
            for b in range(NB):
                T = Tm[b]
                g = g_sb[b]
                mk = mk_sb[b]

                # w2 = exp(prelu(asrc + adst)) pair-broadcast, on ACT
                et = sm.tile([P, T * H], dt.float32, tag="et")
                if layer == 1:
                    adst_in = adst_ps[b][:].rearrange("p (t h) -> p t h", h=H)
                else:
                    adst_in = adste_sb[:, toff[b]:toff[b] + T].rearrange(
                        "p (t h) -> p t h", h=H)
                nc.vector.tensor_tensor(
                    et[:].rearrange("p (t h) -> p t h", h=H),
                    g[:, :, COUT:COUT + 2 * H].bitcast(dt.float32),
                    adst_in,
                    mybir.AluOpType.add)
                adst_ps[b] = None
                lr = sm.tile([P, T * H], dt.float32, tag="lr")
                nc.scalar.activation(lr[:], et[:],
                                     mybir.ActivationFunctionType.Prelu,
                                     alpha=NEG)
                wt2 = sm.tile([P, T, H, 2], dt.bfloat16, tag="wt2")
                nc.scalar.activation(
                    wt2[:],
                    lr[:].rearrange("p (t h o) -> p t h o", h=H, o=1
                                    ).broadcast_to([P, T, H, 2]),
                    mybir.ActivationFunctionType.Exp)

                # lookahead issues (fill the ACT round-trip gap on DVE/Pool;
                # the dlr replication goes last so it never delays a gather)
                if not ALATE and b + GLA < NB:
                    issue_gatherA(b + GLA)
                if b + GLB < NB:
                    issue_gatherB(b + GLB)
                if pend is not None and not EPL:
                    epilogue(*pend)
                    pend = None
                if b + GLA < NB:
                    if ALATE:
                        issue_gatherA(b + GLA)
                    build_masks(b + GLA)
                if b + LKP < NB:
                    prepass_block(b + LKP)

                # hp = [w * h | w]  (pair-packed 2x multiply, split in halves
                # so aggregation can start on the first half early)
                hp = sm.tile([P, T, AGC], dt.bfloat16, tag="hp")
                agg = psB.tile([P, AGC], dt.float32, tag="agg")
                nhs = min(HPS, T)
                bnds = [t * T // nhs for t in range(nhs)] + [T]
                for t0, t1 in zip(bnds[:-1], bnds[1:]):
                    if t0 >= t1:
                        continue
                    ts = t1 - t0
                    nc.vector.tensor_tensor(
                        hp[:, t0:t1, 0:COUT].rearrange(
                            "p t (h f2 pr) -> p t h f2 pr", h=H, pr=2),
                        g[:, t0:t1, 0:COUT].rearrange(
                            "p t (h f2 pr) -> p t h f2 pr", h=H, pr=2),
                        wt2[:, t0:t1].rearrange(
                            "p t (h1 h) pr -> p t h h1 pr", h1=1
                        ).broadcast_to([P, ts, H, F // 2, 2]),
                        mybir.AluOpType.mult)
                    nc.scalar.activation(
                        hp[:, t0:t1, COUT:AGC],
                        wt2[:, t0:t1, :, 0],
                        mybir.ActivationFunctionType.Copy)
                    for t in range(t0, t1):
                        nc.tensor.matmul(agg[:], lhsT=mk[:, t, :],
                                         rhs=hp[:, t, :],
                                         start=(t == 0), stop=(t == T - 1))
                g_sb[b] = None
                mk_sb[b] = None
                if pend is not None and EPL:
                    epilogue(*pend)
                pend = (b, agg)
            epilogue(*pend)
            if layer == 1:
                nc.sync.dma_start(outA2[:], a2all[:])

    nc.compile()
    return nc


# ----------------------------------------------------------------------------
# host-side weight packing
# ----------------------------------------------------------------------------

def _expand_att(att, H, F):
    out = np.zeros((H * F, H), np.float32)
    for h in range(H):
        out[h * F:(h + 1) * F, h] = att[h]
    return out


def _inputs_layer(meta, idx_all, dstl_pt, dstl_row, b, layer):
    H = H1 if layer == 1 else 1
    COUT = H * F1
    b_np = np.asarray(b, np.float32).reshape(1, COUT)
    ior = np.arange(P, dtype=np.float32).reshape(1, P).astype(BF16)
    ioc = np.arange(P, dtype=np.float32).reshape(P, 1)
    in_maps = []
    for m in range(NCORES):
        entry = {
            "idxT": idx_all[m],
            "dstlT": dstl_pt[m].astype(BF16),
            "brow": b_np, "iot_r": ior, "iot_c": ioc,
        }
        if layer == 1:
            entry["dstlR"] = dstl_row[m]
        in_maps.append(entry)
    return in_maps


# ----------------------------------------------------------------------------
# entry point
# ----------------------------------------------------------------------------

def kernel(x, edge_index, W1, att_src1, att_dst1, b1, W2, att_src2, att_dst2,
           b2):
    x = np.asarray(x, np.float32)
    edge_index = np.asarray(edge_index)

    meta, idx_all, dstl_pt, dstl_row = _prep_edges(edge_index)
    asg = np.asarray(meta["asg"])                     # [NB, NCORES]

    # ---- launch 0: per-node projection [h | asrc | adst] ----
    key0 = (0,)
    if key0 not in _prog_cache:
        _prog_cache[key0] = _build_proj()
    nc0 = _prog_cache[key0]

    W1f = np.asarray(W1, np.float32)
    wasrc1 = W1f @ _expand_att(np.asarray(att_src1, np.float32), H1, F1)
    wadst1 = W1f @ _expand_att(np.asarray(att_dst1, np.float32), H1, F1)
    wr_np = np.concatenate([W1f, wasrc1, wadst1], axis=1)
    wr_np = np.ascontiguousarray(wr_np.reshape(1, P, 256 + 2 * H1)).astype(BF16)

    xpad = np.zeros((NCORES * SHARD, IN), np.float32)
    xpad[:N] = x
    in_maps0 = []
    for m in range(NCORES):
        shard = xpad[m * SHARD:(m + 1) * SHARD]
        xs_np = np.ascontiguousarray(
            shard.reshape(NB, P, 1, P).transpose(2, 3, 0, 1)).astype(BF16)
        in_maps0.append({"xs": xs_np, "wr": wr_np})
    res0 = run_bass_kernel_spmd(nc0, in_maps0, list(range(NCORES))).results

    COUT1 = H1 * F1
    htab1 = np.zeros((NPAD, 384), BF16)
    adst1 = np.zeros((N + P, H1), np.float32)         # per-node adst (layer 1)
    for m in range(NCORES):
        lo = m * SHARD
        hi = min(NPAD, (m + 1) * SHARD)
        hts = res0[m]["hts"][:hi - lo]
        htab1[lo:hi, 0:COUT1 + 2 * H1] = hts[:, 0:COUT1 + 2 * H1]
        adst1[lo:hi] = hts[:, COUT1 + 2 * H1:].copy().view(np.float32)

    # per-core adst in assigned-block order: [P, NB*H] (partition = dst local)
    def adst_input(adst_n, H):
        out = []
        for m in range(NCORES):
            a = np.zeros((NB, P, H), np.float32)
            for i in range(NB):
                g = asg[i, m]
                rows = adst_n[g * P:(g + 1) * P]
                a[i, :len(rows)] = rows
            out.append(np.ascontiguousarray(
                a.transpose(1, 0, 2).reshape(P, NB * H)).astype(BF16))
        return out

    # ---- layer 1 ----
    tkey = tuple(meta["Tm"])
    zb1 = bool(np.all(np.asarray(b1) == 0))
    key1 = (1, tkey, zb1)
    if key1 not in _prog_cache:
        _prog_cache[key1] = _build_layer(meta, 1, zb1)
    ncA = _prog_cache[key1]
    in_maps = _inputs_layer(meta, idx_all, dstl_pt, dstl_row, b1, 1)
    W2f = np.asarray(W2, np.float32)
    wasrc2 = W2f @ np.asarray(att_src2, np.float32).reshape(F1, 1)
    wadst2 = W2f @ np.asarray(att_dst2, np.float32).reshape(F1, 1)
    w2r_np = np.ascontiguousarray(
        np.concatenate([W2f, wasrc2, wadst2], axis=1).reshape(2, P, 66)
    ).astype(BF16)
    ident = np.eye(P, dtype=np.float32).astype(BF16)
    a1in = adst_input(adst1, H1)
    for m, mmap in enumerate(in_maps):
        mmap["w2r"] = w2r_np
        mmap["identT"] = ident
        mmap["htab"] = htab1
        mmap["adstT"] = a1in[m]
    resA = run_bass_kernel_spmd(ncA, in_maps, list(range(NCORES))).results

    # reassemble layer-2 table from assigned-block outputs
    htab2 = np.zeros((NPAD, 128), BF16)
    for m in range(NCORES):
        o2 = resA[m]["outT2"]
        for i in range(NB):
            g = asg[i, m]
            if g * P >= NPAD:
                continue
            hi = min(NPAD, (g + 1) * P) - g * P
            htab2[g * P:g * P + hi, 0:66] = o2[i * P:i * P + hi, 0:66]

    # ---- layer 2 ----
    zb2 = bool(np.all(np.asarray(b2) == 0))
    key2 = (2, tkey, zb2)
    if key2 not in _prog_cache:
        _prog_cache[key2] = _build_layer(meta, 2, zb2)
    ncB = _prog_cache[key2]
    in_maps2 = _inputs_layer(meta, idx_all, dstl_pt, dstl_row, b2, 2)
    for m, mmap in enumerate(in_maps2):
        mmap["htab"] = htab2
        mmap["adsteT"] = resA[m]["outA2"]
    resB = run_bass_kernel_spmd(ncB, in_maps2, list(range(NCORES))).results

    out = np.zeros((N, F1), np.float32)
    for m in range(NCORES):
        o = resB[m]["outT"]
        for i in range(NB):
            g = asg[i, m]
            lo = g * P
            if lo >= N:
                continue
            hi = min(N, lo + P)
            out[lo:hi] = o[i * P:i * P + (hi - lo)].astype(np.float32)
    return out
